# revision 11
# baseline (speedup 1.0000x reference)
"""Causal self-attention on 8 Trainium2 NeuronCores (Bass/Tile).

Problem: y = CausalSelfAttention(x; Wq, Wk, Wv, Wo) with
B=4, S=2048, E=1024, H=16 heads of 64, fp32 inputs/outputs.

Sharding (tensor-parallel x data-parallel): core c of 8 handles batch
b = c//2 and head-group g = c%2 (8 of 16 heads). Each core receives
x[b] [S, E], the head-group's columns of Wq/Wk/Wv [E, 512] and rows of
Wo [512, E], and produces a partial output projection [S, E]. The host
sums the two partials per batch.

Per-core dataflow (attention-path operands bf16, accumulation fp32).
All matmuls are issued as PE-tile-packed instruction groups, which run
concurrently on disjoint 64-row / 64-col groups of the 128x128 array
(HW-measured ~218ns per packed pair at N=512 vs ~270ns for one full
matmul):
  xT = transpose(x) via the 2-byte XBAR DMA transpose, in 4 S-chunks
  qT/kT [512, S] = W.T @ x.T and v [S, 512] = x @ Wv, every projection
    matmul col-split into two concurrent M=64 instructions
  per head-pair t, q-chunk of 512, k-tile of 128:
    ST [128, 2x512] = K @ Q.T   (two heads row-packed: disjoint 64-row
                                 groups, concurrent)
    PT = exp(ST/8)              (one wide ACTIVATE per pair; causal mask
                                 = triangular-mask multiply on the
                                 diagonal subblock, both heads in one
                                 strided DVE op)
    AV [128, 512] += [V_h0|V_h1] col-packed PV pair (M=64+64, own
                                 streams); SM [128, 512] += ones-matmul
                                 pair computing broadcast softmax sums
  attT = AV * reciprocal(SM)    (two DVE ops per (pair, q-chunk))
  tail: out = sum_t attT_t.T @ Wo_t, accumulated over all 4 pairs in
    PSUM (col-packed), evacuated once, DMA'd out.

Projections for pair t+1 are emitted between the attention q-chunks of
pair t so the PE never idles and the scalar engine's EXP stream starts
~15us into the kernel instead of after all projections.

No device collectives; the host slices inputs and sums partials.
"""

import numpy as np

import concourse.bass as bass
import concourse.mybir as mybir
from concourse import bacc
from concourse.tile import TileContext

FP = mybir.dt.float32
BF = mybir.dt.bfloat16
P = 128


def build(S=2048, E=1024, HPC=8, DH=64, NQ=512):
    GD = HPC * DH          # 512 head dims per core
    KT_E = E // P          # 8 contraction tiles over E
    ST_S = S // P          # 16 sequence tiles
    QC = S // NQ           # 4 q-chunks
    DT = GD // P           # 4 head pairs
    QSUB = NQ // P         # 4 k-tiles per q-chunk step

    assert DH == 64 and NQ % P == 0 and S % NQ == 0 and E % P == 0

    nc = bacc.Bacc(None, target_bir_lowering=False)
    x_d = nc.dram_tensor("x", [S, E], BF, kind="ExternalInput")
    wq_d = nc.dram_tensor("wq", [E, GD], BF, kind="ExternalInput")
    wk_d = nc.dram_tensor("wk", [E, GD], BF, kind="ExternalInput")
    wv_d = nc.dram_tensor("wv", [E, GD], BF, kind="ExternalInput")
    wo_d = nc.dram_tensor("wo", [GD, E], BF, kind="ExternalInput")
    out_d = nc.dram_tensor("out", [S, E], FP, kind="ExternalOutput")

    with TileContext(nc) as tc:
        with (
            tc.tile_pool(name="consts", bufs=1) as consts,
            tc.tile_pool(name="data", bufs=1) as data,
            tc.tile_pool(name="xT", bufs=1) as xT_pool,
            tc.tile_pool(name="wbuf", bufs=1) as wbuf,
            tc.tile_pool(name="pt_sb", bufs=8) as pt_pool,
            tc.tile_pool(name="rec_sb", bufs=2) as rec_pool,
            tc.tile_pool(name="po_sb", bufs=2) as posb_pool,
            tc.tile_pool(name="st_psum", bufs=2, space="PSUM") as st_pool,
            tc.tile_pool(name="av_psum", bufs=2, space="PSUM") as av_pool,
            tc.tile_pool(name="sm_psum", bufs=2, space="PSUM") as sm_pool,
        ):
            # ---- constants -------------------------------------------------
            ones64 = consts.tile([P, 64], BF)
            nc.vector.memset(ones64[:], 1.0)
            # upper-triangular-inclusive multiplicative mask (valid k <= q),
            # replicated side by side for the two packed heads
            ut = consts.tile([P, P], BF)
            nc.gpsimd.memset(ut[:], 0.0)
            nc.gpsimd.affine_select(
                out=ut[:], in_=ut[:],
                compare_op=mybir.AluOpType.is_gt, fill=1.0,
                base=0, pattern=[[-1, P]], channel_multiplier=1,
            )
            ut2 = consts.tile([P, 2 * P], BF)
            nc.vector.tensor_copy(ut2[:, 0:P], ut[:])
            nc.vector.tensor_copy(ut2[:, P : 2 * P], ut[:])
            # preload the EXP table set while DMAs are in flight
            dummy_in = consts.tile([1, 1], FP)
            nc.vector.memset(dummy_in[:], 0.0)
            dummy_out = consts.tile([1, 1], BF)
            nc.scalar.activation(
                dummy_out[:], dummy_in[:], mybir.ActivationFunctionType.Exp,
                scale=1.0,
            )
            # HAM warmup: keep the PE busy while input DMAs land so the
            # clock gate opens before the first real matmul
            warm_ps = st_pool.tile([P, 2 * NQ], FP, tag="st", name="warm")
            for i in range(40):
                nc.tensor.matmul(
                    warm_ps[0:64, 0:256], lhsT=ones64[:, 0:64],
                    rhs=ut2[:], start=True, stop=True, skip_group_check=True,
                )

            # ---- persistent SBUF data -------------------------------------
            kT = [data.tile([P, S], BF, tag=f"kT{t}", name=f"kT{t}") for t in range(DT)]
            qT = [data.tile([P, S], BF, tag=f"qT{t}", name=f"qT{t}") for t in range(DT)]
            v = [data.tile([P, GD], BF, tag=f"v{st}", name=f"v{st}") for st in range(ST_S)]
            attT = [data.tile([P, S], BF, tag=f"attT{t}", name=f"attT{t}") for t in range(DT)]
            xTc = [
                [xT_pool.tile([P, NQ], BF, tag=f"xT{et}_{sc}", name=f"xT{et}_{sc}")
                 for sc in range(QC)]
                for et in range(KT_E)
            ]
            wv_sb = [wbuf.tile([P, GD], BF, tag=f"wv{kt}", name=f"wv{kt}") for kt in range(KT_E)]
            wo_sb = [wbuf.tile([P, E], BF, tag=f"wo{t}", name=f"wo{t}") for t in range(DT)]

            def emit_weight_loads():
                for kt in range(KT_E):
                    nc.sync.dma_start(out=wv_sb[kt][:], in_=wv_d[kt * P : (kt + 1) * P, :])
                for t in range(DT):
                    nc.sync.dma_start(out=wo_sb[t][:], in_=wo_d[t * P : (t + 1) * P, :])

            def load_kq_weights(mt):
                wk_sb = [
                    wbuf.tile([P, P], BF, tag=f"wk{kt}", name=f"wk{mt}_{kt}", bufs=2)
                    for kt in range(KT_E)
                ]
                wq_sb = [
                    wbuf.tile([P, P], BF, tag=f"wq{kt}", name=f"wq{mt}_{kt}", bufs=2)
                    for kt in range(KT_E)
                ]
                for kt in range(KT_E):
                    nc.sync.dma_start(
                        out=wk_sb[kt][:],
                        in_=wk_d[kt * P : (kt + 1) * P, mt * P : (mt + 1) * P],
                    )
                    nc.sync.dma_start(
                        out=wq_sb[kt][:],
                        in_=wq_d[kt * P : (kt + 1) * P, mt * P : (mt + 1) * P],
                    )
                return wk_sb, wq_sb

            kq_w = []

            def emit_transposes(sc):
                for et in range(KT_E):
                    nc.sync.dma_start(
                        out=xTc[et][sc][:],
                        in_=x_d[sc * NQ : (sc + 1) * NQ, et * P : (et + 1) * P],
                        transpose=True,
                    )

            def emit_proj_kq(mt, nsc):
                """kT[mt]/qT[mt] columns for sequence chunk nsc.

                k and q get separate st-ring tiles so their PSUM->SBUF
                casts release ring slots early (keeps the QK allocations
                of the next attention unit from stalling on the ring).
                """
                wk_sb, wq_sb = kq_w[mt]
                for w_sb, dstT in ((wk_sb, kT), (wq_sb, qT)):
                    ps = st_pool.tile([P, 2 * NQ], FP, tag="st", name=f"pj{mt}_{nsc}")
                    for kt in range(KT_E):
                        for ch in range(2):
                            nc.tensor.matmul(
                                ps[64 * ch : 64 * ch + 64, 0:NQ],
                                lhsT=w_sb[kt][:, 64 * ch : 64 * ch + 64],
                                rhs=xTc[kt][nsc][:],
                                start=(kt == 0), stop=(kt == KT_E - 1),
                                skip_group_check=True,
                            )
                    nc.vector.tensor_copy(dstT[mt][:, nsc * NQ : (nsc + 1) * NQ], ps[:, 0:NQ])

            def emit_proj_v(sta):
                """v tiles sta, sta+1 (natural layout [s, gd])."""
                sc, r0 = divmod(sta * P, NQ)
                for j in range(2):
                    r = r0 + j * P
                    ps = st_pool.tile([P, 2 * NQ], FP, tag="st", name=f"pv{sta + j}")
                    for kt in range(KT_E):
                        for ch in range(2):
                            nc.tensor.matmul(
                                ps[64 * ch : 64 * ch + 64, 0:NQ],
                                lhsT=xTc[kt][sc][:, r + 64 * ch : r + 64 * ch + 64],
                                rhs=wv_sb[kt][:],
                                start=(kt == 0), stop=(kt == KT_E - 1),
                                skip_group_check=True,
                            )
                    nc.vector.tensor_copy(v[sta + j][:], ps[:, 0:NQ])

            def attn_unit(t, qj):
                n_tiles = QSUB * qj + QSUB
                kmax = n_tiles - 1
                av = av_pool.tile([P, NQ], FP, tag="av", name=f"av{t}_{qj}")
                sm = sm_pool.tile([P, NQ], FP, tag="sm", name=f"sm{t}_{qj}")

                def qk(ki):
                    stp = st_pool.tile([P, 2 * NQ], FP, tag="st")
                    d = ki - QSUB * qj
                    off = P * d if d > 0 else 0
                    for half in range(2):
                        pr = 64 * half
                        nc.tensor.matmul(
                            stp[:, half * NQ + off : (half + 1) * NQ],
                            lhsT=kT[t][pr : pr + 64, ki * P : (ki + 1) * P],
                            rhs=qT[t][pr : pr + 64, qj * NQ + off : (qj + 1) * NQ],
                            start=True, stop=True,
                        )
                    return stp, off, d

                def exp_mask(stp, off, d):
                    pt = pt_pool.tile([P, 2 * NQ], BF, tag="pt")
                    if off == 0:
                        nc.scalar.activation(
                            pt[:, 0 : 2 * NQ], stp[:, 0 : 2 * NQ],
                            mybir.ActivationFunctionType.Exp, scale=0.125,
                        )
                    else:
                        # one ACTIVATE over both heads' valid spans via a
                        # strided AP; dead cols are never read downstream
                        pt2 = pt.rearrange("p (k c) -> p k c", c=NQ)
                        st2 = stp.rearrange("p (k c) -> p k c", c=NQ)
                        nc.scalar.activation(
                            pt2[:, :, off:NQ], st2[:, :, off:NQ],
                            mybir.ActivationFunctionType.Exp, scale=0.125,
                        )
                    if d >= 0:
                        # causal mask on the diagonal subblock, both heads in
                        # one strided op
                        pt3 = pt.rearrange("p (k c) -> p k c", c=NQ)
                        ut3 = ut2.rearrange("p (k c) -> p k c", c=P)
                        nc.vector.tensor_tensor(
                            pt3[:, :, off : off + P], pt3[:, :, off : off + P],
                            ut3[:], mybir.AluOpType.mult,
                        )
                    return pt

                def pv_sums(pt, off, ki):
                    st_f, sp_f = (ki == 0), (ki == kmax)
                    for half in range(2):
                        h = 2 * t + half
                        nc.tensor.matmul(
                            av[64 * half : 64 * half + 64, off:NQ],
                            lhsT=v[ki][:, h * DH : h * DH + DH],
                            rhs=pt[:, half * NQ + off : (half + 1) * NQ],
                            start=st_f, stop=sp_f, skip_group_check=True,
                        )
                        nc.tensor.matmul(
                            sm[64 * half : 64 * half + 64, off:NQ],
                            lhsT=ones64[:],
                            rhs=pt[:, half * NQ + off : (half + 1) * NQ],
                            start=st_f, stop=sp_f, skip_group_check=True,
                        )

                # ki-pairs keep same-shape instruction streaks on the PE
                for kp in range(n_tiles // 2):
                    kis = (2 * kp, 2 * kp + 1)
                    sts = [qk(ki) for ki in kis]
                    pts = [exp_mask(stp, off, d) for stp, off, d in sts]
                    for ki, pt, (stp, off, d) in zip(kis, pts, sts):
                        pv_sums(pt, off, ki)

                rec = rec_pool.tile([P, NQ], FP, tag="rec")
                nc.vector.reciprocal_approx_fast(rec[:], sm[:])
                nc.vector.tensor_tensor(
                    attT[t][:, qj * NQ : (qj + 1) * NQ], av[:], rec[:],
                    mybir.AluOpType.mult,
                )

            def emit_outproj(st):
                po = st_pool.tile([P, 2 * NQ], FP, tag="st", name=f"po{st}")
                for nj in range(2):
                    for t in range(DT):
                        for ch in range(2):
                            nc.tensor.matmul(
                                po[64 * ch : 64 * ch + 64, nj * NQ : (nj + 1) * NQ],
                                lhsT=attT[t][:, st * P + 64 * ch : st * P + 64 * ch + 64],
                                rhs=wo_sb[t][:, nj * NQ : (nj + 1) * NQ],
                                start=(t == 0), stop=(t == DT - 1),
                                skip_group_check=True,
                            )
                posb = posb_pool.tile([P, E], FP, tag="posb")
                nc.vector.tensor_copy(posb[:], po[:])
                nc.sync.dma_start(out=out_d[st * P : (st + 1) * P, :], in_=posb[:])

            # ---- main pipeline --------------------------------------------
            # transposes first: the XBAR transpose engine is a serial
            # resource (~42us for all of x) and gates the whole pipeline
            for sc in range(QC):
                emit_transposes(sc)
            emit_weight_loads()
            kq_w.append(load_kq_weights(0))
            kq_w.append(load_kq_weights(1))
            for t in range(DT):
                if 0 < t < DT - 1:
                    kq_w.append(load_kq_weights(t + 1))
                for qj in range(QC):
                    if t == 0:
                        emit_proj_kq(0, qj)
                        emit_proj_v(4 * qj)
                        emit_proj_v(4 * qj + 2)
                        emit_proj_kq(1, qj)
                    elif t < DT - 1:
                        emit_proj_kq(t + 1, qj)
                    attn_unit(t, qj)
                    if t == DT - 1:
                        # output projection (PSUM-accumulated over all 4
                        # pairs) interleaves with the last pair's attention:
                        # fills the PE while the scalar engine works ahead
                        for st in range(4 * qj, 4 * qj + 4):
                            emit_outproj(st)

    nc.compile()
    return nc


_NC_CACHE = {}


def _get_nc():
    if "nc" not in _NC_CACHE:
        _NC_CACHE["nc"] = build()
    return _NC_CACHE["nc"]


B, S, E, H, DH = 4, 2048, 1024, 16, 64
GD = (H // 2) * DH  # 512 per-core head dims


def _in_maps(x, Wq, Wk, Wv, Wo):
    import ml_dtypes

    bf = ml_dtypes.bfloat16
    maps = []
    for c in range(8):
        b, g = c // 2, c % 2
        sl = slice(g * GD, (g + 1) * GD)
        maps.append({
            "x": x[b].astype(bf),
            "wq": Wq[:, sl].astype(bf),
            "wk": Wk[:, sl].astype(bf),
            "wv": Wv[:, sl].astype(bf),
            "wo": Wo[sl, :].astype(bf),
        })
    return maps


def kernel(x, Wq, Wk, Wv, Wo):
    from concourse.bass_utils import run_bass_kernel_spmd

    x = np.asarray(x, dtype=np.float32)
    Wq = np.asarray(Wq, dtype=np.float32)
    Wk = np.asarray(Wk, dtype=np.float32)
    Wv = np.asarray(Wv, dtype=np.float32)
    Wo = np.asarray(Wo, dtype=np.float32)

    res = run_bass_kernel_spmd(
        _get_nc(), _in_maps(x, Wq, Wk, Wv, Wo), list(range(8))
    )

    out = np.empty((B, S, E), np.float32)
    for b in range(B):
        out[b] = res.results[2 * b]["out"] + res.results[2 * b + 1]["out"]
    return out
